# revision 31
# baseline (speedup 1.0000x reference)
"""Trainium2 Bass kernel for nn_CatEdgeGraphLayer.

Reference computation (B=128, N=64, D=128, OUT=128):
    f_i = af[:, :, None, :], f_j = af[:, None, :, :]
    msg = A[..., None] * cat(f_j, diff)              # [B,N,N,D+2]
    inp = cat(f_i, msg)                              # [B,N,N,2D+2]
    h   = inp @ W.T + b                              # [B,N,N,OUT]
    out = relu(sum_{j != i} h[:, :, j, :])           # [B,N,OUT]
    returns (diff, out)

By linearity of the edge Linear over the concat, with W = [W_i | W_j | W_d]
(cols 0:D, D:2D, 2D:2D+2) and Atilde = A with zeroed diagonal:

    out[b,i] = relu( (N-1) * (W_i @ af[b,i] + b)         # "u-term"
                   + W_j @ (Atilde[b] @ af[b])[i]        # "h-term"
                   + W_d @ (sum_j Atilde[b,i,j] * diff[b,i,j,:]) )  # "d-term"

so the [B,N,N,2D+2] edge tensor never needs to be materialized.

Sharding: data-parallel over B across 8 NeuronCores (16 batches/core).

Device layout is fully transposed (out index o on partitions, (b,i) on the
free dim):
  - u-term: batched 512-col matmuls on a host-pretransposed 63*af^T,
    split hi/lo into two bf16 planes (W_i likewise), computed as
    Whi@ahi + Whi@alo + Wlo@ahi -> ~1e-5 relative accuracy at bf16 speed.
  - stage 1: h^T = af^T x Atilde^T, two batches per matmul: the
    stationary operand stacks [af[2p]; af[2p+1]] on the 128 partitions and
    the moving operand is block-diagonal [[At^T[2p], 0], [0, At^T[2p+1]]],
    so the zero blocks kill the cross-batch terms. 8 matmuls total; one
    PSUM bank per 8 batches, copied once (rounding to the stage-2 dtype).
  - stage 2: h-term and d-term as batched 512-col matmuls accumulating
    into the u-term PSUM; the d-term moving operand is
    (Atilde expanded over k) * diff^T, one vector multiply per half.
  - epilogue: relu + per-partition bias (63*b) ACT instructions per
    quarter, each followed by its output DMA.

Scheduling notes (from NTFF traces):
  - ~7us fixed BSP/engine preamble before any instruction runs; DMA
    issue costs ~0.65us of sequencer time each; HBM completion receipt
    adds ~1.5-2.5us per transfer.
  - The two HWDGE rings (sync, scalar) fair-share HBM bandwidth
    packet-by-packet, so a tensor completes only when everything
    co-scheduled with it completes: big tensors are split in half across
    the two rings in strict need-order, constants are packed into a
    single DMA (bitcast views per dtype), and in2 rides the vector
    engine's queue.
  - The PE clock is HAM-gated at 1.2 GHz until ~3.4us of sustained
    activity: dummy matmuls on a zeroed scratch tile (big ones to
    release the gate, small ones to hold it) keep it busy through the
    first-DMA wait so real work runs at 2.4 GHz.
"""

import sys

for _p in ("/opt/trn_rl_repo",):
    if _p not in sys.path:
        sys.path.insert(0, _p)

import numpy as np

B, N, D, OUT = 128, 64, 128, 128
NCORES = 8
BL = B // NCORES  # batches per core
H = BL // 2 * N  # 512: free-dim half (8 batches)

# dtype knobs:
#   ST1: "bfloat16" | "float32r" - stage-1 h^T matmuls + stage-2 h-term
#   UPATH: "hilo" (2-plane bf16, ~1e-5) | "f32r" (single plane, ~1.6e-4)
#   DT2: a2t/difft upload dtype (d-term operands)
ST1 = "float32r"
UPATH = "hilo"
DT2 = "bfloat16"
PREWARM_MMS = 6  # big (512-col) dummies: release the HAM clock gate
PREWARM_SMALL = 14  # small (128-col) dummies: stay warm until data lands
PREWARM_MID = 10  # dummies between stage 1 and stage 2: the aft/in2 DMA
# wait is ~3.5us, long enough for the HAM gate to re-throttle; these hold
# the PE busy so stage 2 runs at 2.4 GHz

_cache = {}


def _build_nc():
    """Build (once) the single-core Bass/Tile program; all 8 cores run it
    SPMD on their own batch shard."""
    key = (ST1, UPATH, DT2, PREWARM_MMS, PREWARM_SMALL, PREWARM_MID)
    if key in _cache:
        return _cache[key]

    from contextlib import ExitStack

    import concourse.mybir as mybir
    import concourse.tile as tile
    from concourse import bacc

    f32 = mybir.dt.float32
    f32r = mybir.dt.float32r
    bf16 = mybir.dt.bfloat16
    u32 = mybir.dt.uint32
    dt1 = getattr(mybir.dt, ST1)
    dt2 = getattr(mybir.dt, DT2)
    dtu = bf16 if UPATH == "hilo" else f32r
    n_aft = 2 if UPATH == "hilo" else 1
    wjw = OUT // 2 if ST1 == "bfloat16" else OUT  # wj cols in u32 words

    nc = bacc.Bacc("TRN2", target_bir_lowering=False, debug=False, num_devices=NCORES)

    # DRAM I/O (per-core shapes)
    # aft: 63*af^T as [d, (b,i)]; hi plane then (hilo) lo plane
    aft = nc.dram_tensor("aft", [D, n_aft * BL * N], dtu, kind="ExternalInput")
    # in1: per batch-pair p, 256 cols: [af-pair (128, [j2,d]) |
    #      blockdiag Atilde^T-pair (128, [j2,i2])]
    in1 = nc.dram_tensor("in1", [2 * N, BL * D], dt1, kind="ExternalInput")
    # in2: per half h: [a2t_h (H) | difft_h (H)]
    in2 = nc.dram_tensor("in2", [2 * N, 4 * H], dt2, kind="ExternalInput")
    # constants in two packed DMAs: u32-typed (bias, wu, bf16 wj) and
    # f32r-typed (wts, f32r wj) -- the BIR verifier requires f32r matmult
    # operands to come from f32r-typed producers, so those can't ride a
    # bitcast from a u32 DMA.
    bf_wj = ST1 == "bfloat16"
    wpw = 129 + (wjw if bf_wj else 0)
    wfw = OUT + (0 if bf_wj else OUT)
    wpack = nc.dram_tensor("wpack", [D, wpw], u32, kind="ExternalInput")
    wfr = nc.dram_tensor("wfr", [D, wfw], f32r, kind="ExternalInput")
    outT = nc.dram_tensor("outT", [OUT, BL * N], f32, kind="ExternalOutput")

    with tile.TileContext(nc) as tc, ExitStack() as ctx:
        consts = ctx.enter_context(tc.tile_pool(name="consts", bufs=1))
        big = ctx.enter_context(tc.tile_pool(name="big", bufs=1))
        hx_pool = ctx.enter_context(tc.tile_pool(name="hx_ps", bufs=2, space="PSUM"))
        u_pool = ctx.enter_context(tc.tile_pool(name="u_ps", bufs=1, space="PSUM"))

        aft_sb = big.tile([D, n_aft * BL * N], dtu)
        in1_sb = big.tile([2 * N, BL * D], dt1)
        in2_sb = big.tile([2 * N, 4 * H], dt2)
        wd_sb = big.tile([2 * N, BL * N], f32r)
        hx_sb = big.tile([D, BL * N], dt1)
        outT_sb = big.tile([OUT, BL * N], f32)
        wp_sb = consts.tile([D, wpw], u32)
        wfr_sb = consts.tile([D, wfw], f32r)

        bias_ap = wp_sb[:, 0:1].bitcast(f32)
        wu_view = wp_sb[:, 1:129].bitcast(dtu)  # [D, n_aft*OUT]
        if bf_wj:
            wj_view = wp_sb[:, 129 : 129 + wjw].bitcast(dt1)  # [D, OUT]
            wts_view = wfr_sb[:, 0:OUT]
        else:
            wj_view = wfr_sb[:, 0:OUT]
            wts_view = wfr_sb[:, OUT : 2 * OUT]

        if PREWARM_MMS or PREWARM_SMALL:
            warm_pool = ctx.enter_context(
                tc.tile_pool(name="warm_ps", bufs=1, space="PSUM")
            )
            warm_sb = consts.tile([D, 512], bf16)
            warm_ps = warm_pool.tile([D, 512], f32)
            nc.gpsimd.memset(warm_sb[:], 0.0)
            for _ in range(PREWARM_MMS):
                nc.tensor.matmul(
                    warm_ps[:], warm_sb[:, 0:D], warm_sb[:], start=True, stop=True
                )
            for _ in range(PREWARM_SMALL):
                nc.tensor.matmul(
                    warm_ps[:, 0:D],
                    warm_sb[:, 0:D],
                    warm_sb[:, 0:D],
                    start=True,
                    stop=True,
                )

        # DMA issue plan: per-half pipeline order. Half 0's chain
        # (in1c0, wu, aft-hi0, aft-lo0, in2h0) lands first; half 1's data
        # streams while half 0 computes and drains its outputs.
        c1 = BL * D // 2
        BLN = BL * N
        nc.sync.dma_start(in1_sb[:, 0:c1], in1[:, 0:c1])
        nc.scalar.dma_start(wp_sb[:], wpack[:])
        nc.scalar.dma_start(wfr_sb[:], wfr[:])
        if UPATH == "hilo":
            nc.sync.dma_start(aft_sb[:, 0:H], aft[:, 0:H])  # hi plane, h0
            nc.scalar.dma_start(
                aft_sb[:, BLN : BLN + H], aft[:, BLN : BLN + H]
            )  # lo plane, h0
            nc.scalar.dma_start(aft_sb[:, H:BLN], aft[:, H:BLN])  # hi plane, h1
            nc.scalar.dma_start(aft_sb[:, BLN + H :], aft[:, BLN + H :])  # lo, h1
        else:
            nc.scalar.dma_start(aft_sb[:, 0:H], aft[:, 0:H])
            nc.scalar.dma_start(aft_sb[:, H:BLN], aft[:, H:BLN])
        nc.sync.dma_start(in1_sb[:, c1:], in1[:, c1:])
        nc.gpsimd.dma_start(in2_sb[:, 0 : 2 * H], in2[:, 0 : 2 * H])
        nc.gpsimd.dma_start(in2_sb[:, 2 * H :], in2[:, 2 * H :])

        u_ps = [
            u_pool.tile([OUT, H], f32, name=f"u_ps{h}", tag=f"u{h}") for h in range(2)
        ]

        for h in range(2):
            if h == 1:
                # half 1's in1 chunk lands late; dependency-free dummies
                # hold the HAM clock gate open through the wait so the
                # whole half-1 chain runs at 2.4 GHz
                for _ in range(PREWARM_MID):
                    nc.tensor.matmul(
                        warm_ps[:, 0:D],
                        warm_sb[:, 0:D],
                        warm_sb[:, 0:D],
                        start=True,
                        stop=True,
                    )

            # stage 1: 4 batch-pair matmuls into one PSUM bank
            hx_ps = hx_pool.tile([D, H], f32, name=f"hx_ps{h}", tag=f"hx{h}")
            for p in range(h * 4, (h + 1) * 4):
                ps = slice(p * 2 * D, p * 2 * D + D)
                ms = slice(p * 2 * D + D, (p + 1) * 2 * D)
                nc.tensor.matmul(
                    hx_ps[:, (p % 4) * 2 * N : (p % 4 + 1) * 2 * N],
                    in1_sb[:, ps],
                    in1_sb[:, ms],
                    start=True,
                    stop=True,
                )
            s = slice(h * H, (h + 1) * H)
            nc.scalar.copy(hx_sb[:, s], hx_ps[:])  # rounds to dt1

            # the elementwise product feeding this half's d-term matmul
            nc.vector.tensor_mul(
                wd_sb[:, s],
                in2_sb[:, 2 * h * H : (2 * h + 1) * H],
                in2_sb[:, (2 * h + 1) * H : (2 * h + 2) * H],
            )

            # stage 2: u-term + h-term + d-term accumulate into one bank
            nc.tensor.matmul(
                u_ps[h][:], wu_view[:, 0:OUT], aft_sb[:, s], start=True, stop=False
            )
            if UPATH == "hilo":
                nc.tensor.matmul(
                    u_ps[h][:],
                    wu_view[:, OUT : 2 * OUT],
                    aft_sb[:, s],
                    start=False,
                    stop=False,
                )
            nc.tensor.matmul(
                u_ps[h][:], wj_view[:], hx_sb[:, s], start=False, stop=False
            )
            if UPATH == "hilo":
                lo_s = slice(BL * N + h * H, BL * N + (h + 1) * H)
                nc.tensor.matmul(
                    u_ps[h][:],
                    wu_view[:, 0:OUT],
                    aft_sb[:, lo_s],
                    start=False,
                    stop=False,
                )
            nc.tensor.matmul(
                u_ps[h][:], wts_view[:], wd_sb[:, s], start=False, stop=True
            )
            for q in range(2):
                qs = slice(h * H + q * H // 2, h * H + (q + 1) * H // 2)
                nc.scalar.activation(
                    outT_sb[:, qs],
                    u_ps[h][:, q * H // 2 : (q + 1) * H // 2],
                    mybir.ActivationFunctionType.Relu,
                    bias=bias_ap,
                    scale=1.0,
                )
                nc.sync.dma_start(outT[:, qs], outT_sb[:, qs])

    nc.compile()
    _cache[key] = nc
    return nc


def _prep_in_maps(diff_vecs, af, A, W, bvec):
    """Host-side shard + relayout. Returns list of per-core input dicts."""
    import ml_dtypes

    bf16 = ml_dtypes.bfloat16
    eye = np.eye(N, dtype=np.float32)
    At = A * (1.0 - eye)[None]  # zero the diagonal: j == i excluded

    wjt = np.ascontiguousarray(W[:, D : 2 * D].T)  # [d, o]
    wts = np.ascontiguousarray(np.tile(W[:, 2 * D : 2 * D + 2].T, (N, 1)))  # wdbig
    wit = np.ascontiguousarray(W[:, 0:D].T)  # [d, o]
    if UPATH == "hilo":
        wu = np.empty((D, 2 * OUT), bf16)
        wu[:, 0:OUT] = wit.astype(bf16)
        wu[:, OUT : 2 * OUT] = (wit - wu[:, 0:OUT].astype(np.float32)).astype(bf16)
        wu_u32 = np.ascontiguousarray(wu).view(np.uint16).view(np.uint32)
    else:
        wu_u32 = wit.view(np.uint32)
    bias63 = ((N - 1.0) * bvec).astype(np.float32)  # [OUT]

    if ST1 == "bfloat16":
        wj_u32 = np.ascontiguousarray(wjt.astype(bf16)).view(np.uint16).view(np.uint32)
        wpack = np.empty((D, 129 + wj_u32.shape[1]), np.uint32)
        wpack[:, 129:] = wj_u32
        wfr = wts
    else:
        wpack = np.empty((D, 129), np.uint32)
        wfr = np.ascontiguousarray(np.concatenate([wjt, wts], axis=1))
    wpack[:, 0] = bias63.view(np.uint32)
    wpack[:, 1:129] = wu_u32

    ndt1 = bf16 if ST1 == "bfloat16" else np.float32
    ndt2 = bf16 if DT2 == "bfloat16" else np.float32

    in_maps = []
    for c in range(NCORES):
        sl = slice(c * BL, (c + 1) * BL)
        af_l = af[sl]  # [BL, N, D]
        At_l = At[sl]  # [BL, i, j]
        diff_l = diff_vecs[sl]  # [BL, i, j, 2]
        At_jbi = At_l.transpose(2, 0, 1)  # [j, b, i]

        af63T = ((N - 1.0) * af_l).transpose(2, 0, 1).reshape(D, BL * N)  # [d,(b,i)]
        if UPATH == "hilo":
            aft_arr = np.empty((D, 2 * BL * N), bf16)
            aft_arr[:, 0 : BL * N] = af63T.astype(bf16)
            aft_arr[:, BL * N :] = (
                af63T - aft_arr[:, 0 : BL * N].astype(np.float32)
            ).astype(bf16)
        else:
            aft_arr = np.ascontiguousarray(af63T)

        # pair layout: [af[2p]; af[2p+1]] stacked on rows, then
        # block-diag [[At^T[2p], 0], [0, At^T[2p+1]]]
        in1 = np.zeros((2 * N, BL // 2, 2 * D), np.float32)
        afT_jbd = af_l.transpose(1, 0, 2)  # [j, b, d]
        in1[0:N, :, 0:D] = afT_jbd[:, 0::2]
        in1[N : 2 * N, :, 0:D] = afT_jbd[:, 1::2]
        in1[0:N, :, D : D + N] = At_jbi[:, 0::2]
        in1[N : 2 * N, :, D + N : 2 * D] = At_jbi[:, 1::2]

        in2 = np.empty((2 * N, 4 * H), ndt2)
        a2t = np.repeat(At_jbi, 2, axis=0).reshape(2 * N, BL * N)
        difft = diff_l.transpose(2, 3, 0, 1).reshape(2 * N, BL * N)
        for h in range(2):
            in2[:, 2 * h * H : (2 * h + 1) * H] = a2t[:, h * H : (h + 1) * H]
            in2[:, (2 * h + 1) * H : (2 * h + 2) * H] = difft[:, h * H : (h + 1) * H]

        in_maps.append(
            {
                "aft": aft_arr,
                "wpack": wpack,
                "wfr": wfr,
                "in1": np.ascontiguousarray(
                    in1.reshape(2 * N, BL * D).astype(ndt1, copy=False)
                ),
                "in2": in2,
            }
        )
    return in_maps


def _gather(results):
    """[8] x outT[OUT, BL*N] -> out[B, N, OUT]"""
    outT = np.stack([results[c]["outT"] for c in range(NCORES)], axis=0)
    return np.ascontiguousarray(
        outT.reshape(NCORES, OUT, BL, N).transpose(0, 2, 3, 1).reshape(B, N, OUT)
    )


def kernel(**inputs):
    from concourse.bass_utils import run_bass_kernel_spmd

    diff_vecs = np.ascontiguousarray(np.asarray(inputs["diff_vecs"], dtype=np.float32))
    af = np.asarray(inputs["agent_features"], dtype=np.float32)
    A = np.asarray(inputs["A"], dtype=np.float32)
    W = np.asarray(inputs["W"], dtype=np.float32)
    bvec = np.asarray(inputs["b"], dtype=np.float32).reshape(-1)

    nc = _build_nc()
    in_maps = _prep_in_maps(diff_vecs, af, A, W, bvec)
    res = run_bass_kernel_spmd(nc, in_maps, list(range(NCORES))).results
    return diff_vecs, _gather(res)


# revision 33
# speedup vs baseline: 1.0106x; 1.0106x over previous
"""Trainium2 Bass kernel for nn_CatEdgeGraphLayer.

Reference computation (B=128, N=64, D=128, OUT=128):
    f_i = af[:, :, None, :], f_j = af[:, None, :, :]
    msg = A[..., None] * cat(f_j, diff)              # [B,N,N,D+2]
    inp = cat(f_i, msg)                              # [B,N,N,2D+2]
    h   = inp @ W.T + b                              # [B,N,N,OUT]
    out = relu(sum_{j != i} h[:, :, j, :])           # [B,N,OUT]
    returns (diff, out)

By linearity of the edge Linear over the concat, with W = [W_i | W_j | W_d]
(cols 0:D, D:2D, 2D:2D+2) and Atilde = A with zeroed diagonal:

    out[b,i] = relu( (N-1) * (W_i @ af[b,i] + b)         # "u-term"
                   + W_j @ (Atilde[b] @ af[b])[i]        # "h-term"
                   + W_d @ (sum_j Atilde[b,i,j] * diff[b,i,j,:]) )  # "d-term"

so the [B,N,N,2D+2] edge tensor never needs to be materialized.

Sharding: data-parallel over B across 8 NeuronCores (16 batches/core).

Device layout is fully transposed (out index o on partitions, (b,i) on the
free dim):
  - u-term: batched 512-col matmuls on a host-pretransposed 63*af^T,
    split hi/lo into two bf16 planes (W_i likewise), computed as
    Whi@ahi + Whi@alo + Wlo@ahi -> ~1e-5 relative accuracy at bf16 speed.
  - stage 1: h^T = af^T x Atilde^T, two batches per matmul: the
    stationary operand stacks [af[2p]; af[2p+1]] on the 128 partitions and
    the moving operand is block-diagonal [[At^T[2p], 0], [0, At^T[2p+1]]],
    so the zero blocks kill the cross-batch terms. 8 matmuls total; one
    PSUM bank per 8 batches, copied once (rounding to the stage-2 dtype).
  - stage 2: h-term and d-term as batched 512-col matmuls accumulating
    into the u-term PSUM; the d-term moving operand is
    (Atilde expanded over k) * diff^T, one vector multiply per half.
  - epilogue: relu + per-partition bias (63*b) ACT instructions per
    quarter, each followed by its output DMA.

Scheduling notes (from NTFF traces):
  - ~7us fixed BSP/engine preamble before any instruction runs; DMA
    issue costs ~0.65us of sequencer time each; HBM completion receipt
    adds ~1.5-2.5us per transfer.
  - The two HWDGE rings (sync, scalar) fair-share HBM bandwidth
    packet-by-packet, so a tensor completes only when everything
    co-scheduled with it completes: big tensors are split in half across
    the two rings in strict need-order, constants are packed into a
    single DMA (bitcast views per dtype), and in2 rides the vector
    engine's queue.
  - The PE clock is HAM-gated at 1.2 GHz until ~3.4us of sustained
    activity: dummy matmuls on a zeroed scratch tile (big ones to
    release the gate, small ones to hold it) keep it busy through the
    first-DMA wait so real work runs at 2.4 GHz.
"""

import sys

for _p in ("/opt/trn_rl_repo",):
    if _p not in sys.path:
        sys.path.insert(0, _p)

import numpy as np

B, N, D, OUT = 128, 64, 128, 128
NCORES = 8
BL = B // NCORES  # batches per core
H = BL // 2 * N  # 512: free-dim half (8 batches)

# dtype knobs:
#   ST1: "bfloat16" | "float32r" - stage-1 h^T matmuls + stage-2 h-term
#   UPATH: "hilo" (2-plane bf16, ~1e-5) | "f32r" (single plane, ~1.6e-4)
#   DT2: a2t/difft upload dtype (d-term operands)
ST1 = "float32r"
UPATH = "hilo"
DT2 = "bfloat16"
PREWARM_MMS = 6  # big (512-col) dummies: release the HAM clock gate
PREWARM_SMALL = 14  # small (128-col) dummies: stay warm until data lands
PREWARM_MID = 10  # dummies between stage 1 and stage 2: the aft/in2 DMA
# wait is ~3.5us, long enough for the HAM gate to re-throttle; these hold
# the PE busy so stage 2 runs at 2.4 GHz

_cache = {}


def _build_nc():
    """Build (once) the single-core Bass/Tile program; all 8 cores run it
    SPMD on their own batch shard."""
    key = (ST1, UPATH, DT2, PREWARM_MMS, PREWARM_SMALL, PREWARM_MID)
    if key in _cache:
        return _cache[key]

    from contextlib import ExitStack

    import concourse.mybir as mybir
    import concourse.tile as tile
    from concourse import bacc

    f32 = mybir.dt.float32
    f32r = mybir.dt.float32r
    bf16 = mybir.dt.bfloat16
    u32 = mybir.dt.uint32
    dt1 = getattr(mybir.dt, ST1)
    dt2 = getattr(mybir.dt, DT2)
    dtu = bf16 if UPATH == "hilo" else f32r
    n_aft = 2 if UPATH == "hilo" else 1
    wjw = OUT // 2 if ST1 == "bfloat16" else OUT  # wj cols in u32 words

    nc = bacc.Bacc("TRN2", target_bir_lowering=False, debug=False, num_devices=NCORES)

    # DRAM I/O (per-core shapes)
    # aft: 63*af^T as [d, (b,i)]; hi plane then (hilo) lo plane
    aft = nc.dram_tensor("aft", [D, n_aft * BL * N], dtu, kind="ExternalInput")
    # in1: per batch-pair p, 256 cols: [af-pair (128, [j2,d]) |
    #      blockdiag Atilde^T-pair (128, [j2,i2])]
    in1 = nc.dram_tensor("in1", [2 * N, BL * D], dt1, kind="ExternalInput")
    # in2: per half h: [a2t_h (H) | difft_h (H)]
    in2 = nc.dram_tensor("in2", [2 * N, 4 * H], dt2, kind="ExternalInput")
    # constants in two packed DMAs: u32-typed (bias, wu, bf16 wj) and
    # f32r-typed (wts, f32r wj) -- the BIR verifier requires f32r matmult
    # operands to come from f32r-typed producers, so those can't ride a
    # bitcast from a u32 DMA.
    bf_wj = ST1 == "bfloat16"
    wpw = 129 + (wjw if bf_wj else 0)
    wfw = OUT + (0 if bf_wj else OUT)
    wpack = nc.dram_tensor("wpack", [D, wpw], u32, kind="ExternalInput")
    wfr = nc.dram_tensor("wfr", [D, wfw], f32r, kind="ExternalInput")
    outT = nc.dram_tensor("outT", [OUT, BL * N], f32, kind="ExternalOutput")

    with tile.TileContext(nc) as tc, ExitStack() as ctx:
        consts = ctx.enter_context(tc.tile_pool(name="consts", bufs=1))
        big = ctx.enter_context(tc.tile_pool(name="big", bufs=1))
        hx_pool = ctx.enter_context(tc.tile_pool(name="hx_ps", bufs=2, space="PSUM"))
        u_pool = ctx.enter_context(tc.tile_pool(name="u_ps", bufs=1, space="PSUM"))

        aft_sb = big.tile([D, n_aft * BL * N], dtu)
        in1_sb = big.tile([2 * N, BL * D], dt1)
        in2_sb = big.tile([2 * N, 4 * H], dt2)
        wd_sb = big.tile([2 * N, BL * N], f32r)
        hx_sb = big.tile([D, BL * N], dt1)
        outT_sb = big.tile([OUT, BL * N], f32)
        wp_sb = consts.tile([D, wpw], u32)
        wfr_sb = consts.tile([D, wfw], f32r)

        bias_ap = wp_sb[:, 0:1].bitcast(f32)
        wu_view = wp_sb[:, 1:129].bitcast(dtu)  # [D, n_aft*OUT]
        if bf_wj:
            wj_view = wp_sb[:, 129 : 129 + wjw].bitcast(dt1)  # [D, OUT]
            wts_view = wfr_sb[:, 0:OUT]
        else:
            wj_view = wfr_sb[:, 0:OUT]
            wts_view = wfr_sb[:, OUT : 2 * OUT]

        if PREWARM_MMS or PREWARM_SMALL:
            warm_pool = ctx.enter_context(
                tc.tile_pool(name="warm_ps", bufs=1, space="PSUM")
            )
            warm_sb = consts.tile([D, 512], bf16)
            warm_ps = warm_pool.tile([D, 512], f32)
            nc.gpsimd.memset(warm_sb[:], 0.0)
            for _ in range(PREWARM_MMS):
                nc.tensor.matmul(
                    warm_ps[:], warm_sb[:, 0:D], warm_sb[:], start=True, stop=True
                )
            for _ in range(PREWARM_SMALL):
                nc.tensor.matmul(
                    warm_ps[:, 0:D],
                    warm_sb[:, 0:D],
                    warm_sb[:, 0:D],
                    start=True,
                    stop=True,
                )

        # DMA issue plan: per-half pipeline order. Half 0's chain
        # (in1c0, wu, aft-hi0, aft-lo0, in2h0) lands first; half 1's data
        # streams while half 0 computes and drains its outputs.
        c1 = BL * D // 2
        BLN = BL * N
        nc.sync.dma_start(in1_sb[:, 0:c1], in1[:, 0:c1])
        nc.scalar.dma_start(wp_sb[:], wpack[:])
        nc.scalar.dma_start(wfr_sb[:], wfr[:])
        if UPATH == "hilo":
            nc.sync.dma_start(aft_sb[:, 0:H], aft[:, 0:H])  # hi plane, h0
            nc.scalar.dma_start(
                aft_sb[:, BLN : BLN + H], aft[:, BLN : BLN + H]
            )  # lo plane, h0
            nc.scalar.dma_start(aft_sb[:, H:BLN], aft[:, H:BLN])  # hi plane, h1
            nc.scalar.dma_start(aft_sb[:, BLN + H :], aft[:, BLN + H :])  # lo, h1
        else:
            nc.scalar.dma_start(aft_sb[:, 0:H], aft[:, 0:H])
            nc.scalar.dma_start(aft_sb[:, H:BLN], aft[:, H:BLN])
        nc.sync.dma_start(in1_sb[:, c1:], in1[:, c1:])
        nc.gpsimd.dma_start(in2_sb[:, 0 : 2 * H], in2[:, 0 : 2 * H])
        nc.gpsimd.dma_start(in2_sb[:, 2 * H :], in2[:, 2 * H :])

        u_ps = [
            u_pool.tile([OUT, H], f32, name=f"u_ps{h}", tag=f"u{h}") for h in range(2)
        ]

        for h in range(2):
            if h == 1:
                # half 1's in1 chunk lands late; dependency-free dummies
                # hold the HAM clock gate open through the wait so the
                # whole half-1 chain runs at 2.4 GHz
                for _ in range(PREWARM_MID):
                    nc.tensor.matmul(
                        warm_ps[:, 0:D],
                        warm_sb[:, 0:D],
                        warm_sb[:, 0:D],
                        start=True,
                        stop=True,
                    )

            # stage 1: 4 batch-pair matmuls into one PSUM bank
            hx_ps = hx_pool.tile([D, H], f32, name=f"hx_ps{h}", tag=f"hx{h}")
            for p in range(h * 4, (h + 1) * 4):
                ps = slice(p * 2 * D, p * 2 * D + D)
                ms = slice(p * 2 * D + D, (p + 1) * 2 * D)
                nc.tensor.matmul(
                    hx_ps[:, (p % 4) * 2 * N : (p % 4 + 1) * 2 * N],
                    in1_sb[:, ps],
                    in1_sb[:, ms],
                    start=True,
                    stop=True,
                )
            s = slice(h * H, (h + 1) * H)
            nc.scalar.copy(hx_sb[:, s], hx_ps[:])  # rounds to dt1

            # the elementwise product feeding this half's d-term matmul
            nc.vector.tensor_mul(
                wd_sb[:, s],
                in2_sb[:, 2 * h * H : (2 * h + 1) * H],
                in2_sb[:, (2 * h + 1) * H : (2 * h + 2) * H],
            )

            # stage 2: u-term + h-term + d-term accumulate into one bank
            nc.tensor.matmul(
                u_ps[h][:], wu_view[:, 0:OUT], aft_sb[:, s], start=True, stop=False
            )
            if UPATH == "hilo":
                nc.tensor.matmul(
                    u_ps[h][:],
                    wu_view[:, OUT : 2 * OUT],
                    aft_sb[:, s],
                    start=False,
                    stop=False,
                )
            nc.tensor.matmul(
                u_ps[h][:], wj_view[:], hx_sb[:, s], start=False, stop=False
            )
            if UPATH == "hilo":
                lo_s = slice(BL * N + h * H, BL * N + (h + 1) * H)
                nc.tensor.matmul(
                    u_ps[h][:],
                    wu_view[:, 0:OUT],
                    aft_sb[:, lo_s],
                    start=False,
                    stop=False,
                )
            nc.tensor.matmul(
                u_ps[h][:], wts_view[:], wd_sb[:, s], start=False, stop=True
            )
            for q in range(2):
                qs = slice(h * H + q * H // 2, h * H + (q + 1) * H // 2)
                nc.scalar.activation(
                    outT_sb[:, qs],
                    u_ps[h][:, q * H // 2 : (q + 1) * H // 2],
                    mybir.ActivationFunctionType.Relu,
                    bias=bias_ap,
                    scale=1.0,
                )
                nc.sync.dma_start(outT[:, qs], outT_sb[:, qs])

    nc.compile()
    _cache[key] = nc
    return nc


def _prep_in_maps(diff_vecs, af, A, W, bvec):
    """Host-side shard + relayout. Returns list of per-core input dicts."""
    import ml_dtypes

    bf16 = ml_dtypes.bfloat16
    eye = np.eye(N, dtype=np.float32)
    At = A * (1.0 - eye)[None]  # zero the diagonal: j == i excluded

    wjt = np.ascontiguousarray(W[:, D : 2 * D].T)  # [d, o]
    wts = np.ascontiguousarray(np.tile(W[:, 2 * D : 2 * D + 2].T, (N, 1)))  # wdbig
    wit = np.ascontiguousarray(W[:, 0:D].T)  # [d, o]
    if UPATH == "hilo":
        wu = np.empty((D, 2 * OUT), bf16)
        wu[:, 0:OUT] = wit.astype(bf16)
        wu[:, OUT : 2 * OUT] = (wit - wu[:, 0:OUT].astype(np.float32)).astype(bf16)
        wu_u32 = np.ascontiguousarray(wu).view(np.uint16).view(np.uint32)
    else:
        wu_u32 = wit.view(np.uint32)
    bias63 = ((N - 1.0) * bvec).astype(np.float32)  # [OUT]

    if ST1 == "bfloat16":
        wj_u32 = np.ascontiguousarray(wjt.astype(bf16)).view(np.uint16).view(np.uint32)
        wpack = np.empty((D, 129 + wj_u32.shape[1]), np.uint32)
        wpack[:, 129:] = wj_u32
        wfr = wts
    else:
        wpack = np.empty((D, 129), np.uint32)
        wfr = np.ascontiguousarray(np.concatenate([wjt, wts], axis=1))
    wpack[:, 0] = bias63.view(np.uint32)
    wpack[:, 1:129] = wu_u32

    ndt1 = bf16 if ST1 == "bfloat16" else np.float32
    ndt2 = bf16 if DT2 == "bfloat16" else np.float32

    in_maps = []
    for c in range(NCORES):
        sl = slice(c * BL, (c + 1) * BL)
        af_l = af[sl]  # [BL, N, D]
        At_l = At[sl]  # [BL, i, j]
        diff_l = diff_vecs[sl]  # [BL, i, j, 2]
        At_jbi = At_l.transpose(2, 0, 1)  # [j, b, i]

        af63T = ((N - 1.0) * af_l).transpose(2, 0, 1).reshape(D, BL * N)  # [d,(b,i)]
        if UPATH == "hilo":
            aft_arr = np.empty((D, 2 * BL * N), bf16)
            aft_arr[:, 0 : BL * N] = af63T.astype(bf16)
            aft_arr[:, BL * N :] = (
                af63T - aft_arr[:, 0 : BL * N].astype(np.float32)
            ).astype(bf16)
        else:
            aft_arr = np.ascontiguousarray(af63T)

        # pair layout: [af[2p]; af[2p+1]] stacked on rows, then
        # block-diag [[At^T[2p], 0], [0, At^T[2p+1]]]
        in1 = np.zeros((2 * N, BL // 2, 2 * D), np.float32)
        afT_jbd = af_l.transpose(1, 0, 2)  # [j, b, d]
        in1[0:N, :, 0:D] = afT_jbd[:, 0::2]
        in1[N : 2 * N, :, 0:D] = afT_jbd[:, 1::2]
        in1[0:N, :, D : D + N] = At_jbi[:, 0::2]
        in1[N : 2 * N, :, D + N : 2 * D] = At_jbi[:, 1::2]

        in2 = np.empty((2 * N, 4 * H), ndt2)
        a2t = np.repeat(At_jbi, 2, axis=0).reshape(2 * N, BL * N)
        difft = diff_l.transpose(2, 3, 0, 1).reshape(2 * N, BL * N)
        for h in range(2):
            in2[:, 2 * h * H : (2 * h + 1) * H] = a2t[:, h * H : (h + 1) * H]
            in2[:, (2 * h + 1) * H : (2 * h + 2) * H] = difft[:, h * H : (h + 1) * H]

        in_maps.append(
            {
                "aft": aft_arr,
                "wpack": wpack,
                "wfr": wfr,
                "in1": np.ascontiguousarray(
                    in1.reshape(2 * N, BL * D).astype(ndt1, copy=False)
                ),
                "in2": in2,
            }
        )
    return in_maps


def _gather(results):
    """[8] x outT[OUT, BL*N] -> out[B, N, OUT]"""
    outT = np.stack([results[c]["outT"] for c in range(NCORES)], axis=0)
    return np.ascontiguousarray(
        outT.reshape(NCORES, OUT, BL, N).transpose(0, 2, 3, 1).reshape(B, N, OUT)
    )


def kernel(**inputs):
    from concourse.bass_utils import run_bass_kernel_spmd

    diff_vecs = np.ascontiguousarray(np.asarray(inputs["diff_vecs"], dtype=np.float32))
    af = np.asarray(inputs["agent_features"], dtype=np.float32)
    A = np.asarray(inputs["A"], dtype=np.float32)
    W = np.asarray(inputs["W"], dtype=np.float32)
    bvec = np.asarray(inputs["b"], dtype=np.float32).reshape(-1)

    nc = _build_nc()
    in_maps = _prep_in_maps(diff_vecs, af, A, W, bvec)
    res = run_bass_kernel_spmd(nc, in_maps, list(range(NCORES))).results
    return diff_vecs, _gather(res)


# revision 34
# speedup vs baseline: 1.0847x; 1.0733x over previous
"""Trainium2 Bass kernel for nn_CatEdgeGraphLayer.

Reference computation (B=128, N=64, D=128, OUT=128):
    f_i = af[:, :, None, :], f_j = af[:, None, :, :]
    msg = A[..., None] * cat(f_j, diff)              # [B,N,N,D+2]
    inp = cat(f_i, msg)                              # [B,N,N,2D+2]
    h   = inp @ W.T + b                              # [B,N,N,OUT]
    out = relu(sum_{j != i} h[:, :, j, :])           # [B,N,OUT]
    returns (diff, out)

By linearity of the edge Linear over the concat, with W = [W_i | W_j | W_d]
(cols 0:D, D:2D, 2D:2D+2) and Atilde = A with zeroed diagonal:

    out[b,i] = relu( (N-1) * (W_i @ af[b,i] + b)         # "u-term"
                   + W_j @ (Atilde[b] @ af[b])[i]        # "h-term"
                   + W_d @ (sum_j Atilde[b,i,j] * diff[b,i,j,:]) )  # "d-term"

so the [B,N,N,2D+2] edge tensor never needs to be materialized.

Sharding: data-parallel over B across 8 NeuronCores (16 batches/core).

Device layout is fully transposed (out index o on partitions, (b,i) on the
free dim):
  - u-term: batched 512-col matmuls on a host-pretransposed 63*af^T,
    split hi/lo into two bf16 planes (W_i likewise), computed as
    Whi@ahi + Whi@alo + Wlo@ahi -> ~1e-5 relative accuracy at bf16 speed.
  - stage 1: h^T = af^T x Atilde^T, two batches per matmul: the
    stationary operand stacks [af[2p]; af[2p+1]] on the 128 partitions and
    the moving operand is block-diagonal [[At^T[2p], 0], [0, At^T[2p+1]]],
    so the zero blocks kill the cross-batch terms. 8 matmuls total; one
    PSUM bank per 8 batches, copied once (rounding to the stage-2 dtype).
  - stage 2: h-term and d-term as batched 512-col matmuls accumulating
    into the u-term PSUM; the d-term moving operand is
    (Atilde expanded over k) * diff^T, one vector multiply per half.
  - epilogue: relu + per-partition bias (63*b) ACT instructions per
    quarter, each followed by its output DMA.

Scheduling notes (from NTFF traces):
  - ~7us fixed BSP/engine preamble before any instruction runs; DMA
    issue costs ~0.65us of sequencer time each; HBM completion receipt
    adds ~1.5-2.5us per transfer.
  - The two HWDGE rings (sync, scalar) fair-share HBM bandwidth
    packet-by-packet, so a tensor completes only when everything
    co-scheduled with it completes: big tensors are split in half across
    the two rings in strict need-order, constants are packed into a
    single DMA (bitcast views per dtype), and in2 rides the vector
    engine's queue.
  - The PE clock is HAM-gated at 1.2 GHz until ~3.4us of sustained
    activity: dummy matmuls on a zeroed scratch tile (big ones to
    release the gate, small ones to hold it) keep it busy through the
    first-DMA wait so real work runs at 2.4 GHz.
"""

import sys

for _p in ("/opt/trn_rl_repo",):
    if _p not in sys.path:
        sys.path.insert(0, _p)

import numpy as np

B, N, D, OUT = 128, 64, 128, 128
NCORES = 8
BL = B // NCORES  # batches per core
H = BL // 2 * N  # 512: free-dim half (8 batches)

# dtype knobs:
#   ST1: "bfloat16" | "float32r" - stage-1 h^T matmuls + stage-2 h-term
#   UPATH: "hilo" (2-plane bf16, ~1e-5) | "f32r" (single plane, ~1.6e-4)
#   DT2: a2t/difft upload dtype (d-term operands)
ST1 = "float32r"
UPATH = "hilo"
DT2 = "bfloat16"
PREWARM_MMS = 6  # big (512-col) dummies: release the HAM clock gate
PREWARM_SMALL = 14  # small (128-col) dummies: stay warm until data lands
PREWARM_MID = 10  # dummies between stage 1 and stage 2: the aft/in2 DMA
# wait is ~3.5us, long enough for the HAM gate to re-throttle; these hold
# the PE busy so stage 2 runs at 2.4 GHz

_cache = {}


def _build_nc():
    """Build (once) the single-core Bass/Tile program; all 8 cores run it
    SPMD on their own batch shard."""
    key = (ST1, UPATH, DT2, PREWARM_MMS, PREWARM_SMALL, PREWARM_MID)
    if key in _cache:
        return _cache[key]

    from contextlib import ExitStack

    import concourse.mybir as mybir
    import concourse.tile as tile
    from concourse import bacc

    f32 = mybir.dt.float32
    f32r = mybir.dt.float32r
    bf16 = mybir.dt.bfloat16
    u32 = mybir.dt.uint32
    dt1 = getattr(mybir.dt, ST1)
    dt2 = getattr(mybir.dt, DT2)
    dtu = bf16 if UPATH == "hilo" else f32r
    n_aft = 2 if UPATH == "hilo" else 1
    wjw = OUT // 2 if ST1 == "bfloat16" else OUT  # wj cols in u32 words

    nc = bacc.Bacc("TRN2", target_bir_lowering=False, debug=False, num_devices=NCORES)

    # DRAM I/O (per-core shapes)
    # aft: 63*af^T as [d, (b,i)]; hi plane then (hilo) lo plane
    aft = nc.dram_tensor("aft", [D, n_aft * BL * N], dtu, kind="ExternalInput")
    # in1: per batch-pair p, 256 cols: [af-pair (128, [j2,d]) |
    #      blockdiag Atilde^T-pair (128, [j2,i2])]
    in1 = nc.dram_tensor("in1", [2 * N, BL * D], dt1, kind="ExternalInput")
    # in2: (Atilde (x) diff)^T, premultiplied on host: [2j+k, (b,i)]
    in2 = nc.dram_tensor("in2", [2 * N, 2 * H], dt2, kind="ExternalInput")
    # constants in two packed DMAs: u32-typed (bias, wu, bf16 wj) and
    # f32r-typed (wts, f32r wj) -- the BIR verifier requires f32r matmult
    # operands to come from f32r-typed producers, so those can't ride a
    # bitcast from a u32 DMA.
    bf_wj = ST1 == "bfloat16"
    wpw = 129 + OUT // 2 + (wjw if bf_wj else 0)  # + bf16 wts
    wfw = OUT  # f32r wj (unused cols when bf_wj)
    wpack = nc.dram_tensor("wpack", [D, wpw], u32, kind="ExternalInput")
    wfr = nc.dram_tensor("wfr", [D, wfw], f32r, kind="ExternalInput")
    outT = nc.dram_tensor("outT", [OUT, BL * N], f32, kind="ExternalOutput")

    with tile.TileContext(nc) as tc, ExitStack() as ctx:
        consts = ctx.enter_context(tc.tile_pool(name="consts", bufs=1))
        big = ctx.enter_context(tc.tile_pool(name="big", bufs=1))
        hx_pool = ctx.enter_context(tc.tile_pool(name="hx_ps", bufs=2, space="PSUM"))
        u_pool = ctx.enter_context(tc.tile_pool(name="u_ps", bufs=1, space="PSUM"))

        aft_sb = big.tile([D, n_aft * BL * N], dtu)
        in1_sb = big.tile([2 * N, BL * D], dt1)
        in2_sb = big.tile([2 * N, 2 * H], dt2)
        hx_sb = big.tile([D, BL * N], dt1)
        outT_sb = big.tile([OUT, BL * N], f32)
        wp_sb = consts.tile([D, wpw], u32)
        wfr_sb = consts.tile([D, wfw], f32r)

        bias_ap = wp_sb[:, 0:1].bitcast(f32)
        wu_view = wp_sb[:, 1:129].bitcast(dtu)  # [D, n_aft*OUT]
        wts_view = wp_sb[:, 129 : 129 + OUT // 2].bitcast(bf16)  # [D, OUT]
        if bf_wj:
            wj_view = wp_sb[:, 129 + OUT // 2 :].bitcast(dt1)  # [D, OUT]
        else:
            wj_view = wfr_sb[:, 0:OUT]

        if PREWARM_MMS or PREWARM_SMALL:
            warm_pool = ctx.enter_context(
                tc.tile_pool(name="warm_ps", bufs=1, space="PSUM")
            )
            warm_sb = consts.tile([D, 512], bf16)
            warm_ps = warm_pool.tile([D, 512], f32)
            nc.gpsimd.memset(warm_sb[:], 0.0)
            for _ in range(PREWARM_MMS):
                nc.tensor.matmul(
                    warm_ps[:], warm_sb[:, 0:D], warm_sb[:], start=True, stop=True
                )
            for _ in range(PREWARM_SMALL):
                nc.tensor.matmul(
                    warm_ps[:, 0:D],
                    warm_sb[:, 0:D],
                    warm_sb[:, 0:D],
                    start=True,
                    stop=True,
                )

        # DMA issue plan: per-half pipeline order. Half 0's chain
        # (in1c0, wu, aft-hi0, aft-lo0, in2h0) lands first; half 1's data
        # streams while half 0 computes and drains its outputs.
        c1 = BL * D // 2
        BLN = BL * N
        nc.sync.dma_start(in1_sb[:, 0:c1], in1[:, 0:c1])
        nc.scalar.dma_start(wp_sb[:], wpack[:])
        nc.scalar.dma_start(wfr_sb[:], wfr[:])
        if UPATH == "hilo":
            nc.sync.dma_start(aft_sb[:, 0:H], aft[:, 0:H])  # hi plane, h0
            nc.scalar.dma_start(
                aft_sb[:, BLN : BLN + H], aft[:, BLN : BLN + H]
            )  # lo plane, h0
            nc.scalar.dma_start(aft_sb[:, H:BLN], aft[:, H:BLN])  # hi plane, h1
            nc.scalar.dma_start(aft_sb[:, BLN + H :], aft[:, BLN + H :])  # lo, h1
        else:
            nc.scalar.dma_start(aft_sb[:, 0:H], aft[:, 0:H])
            nc.scalar.dma_start(aft_sb[:, H:BLN], aft[:, H:BLN])
        nc.sync.dma_start(in1_sb[:, c1:], in1[:, c1:])
        nc.gpsimd.dma_start(in2_sb[:, 0:H], in2[:, 0:H])
        nc.gpsimd.dma_start(in2_sb[:, H:], in2[:, H:])

        u_ps = [
            u_pool.tile([OUT, H], f32, name=f"u_ps{h}", tag=f"u{h}") for h in range(2)
        ]

        for h in range(2):
            if h == 1:
                # half 1's in1 chunk lands late; dependency-free dummies
                # hold the HAM clock gate open through the wait so the
                # whole half-1 chain runs at 2.4 GHz
                for _ in range(PREWARM_MID):
                    nc.tensor.matmul(
                        warm_ps[:, 0:D],
                        warm_sb[:, 0:D],
                        warm_sb[:, 0:D],
                        start=True,
                        stop=True,
                    )

            # stage 1: 4 batch-pair matmuls into one PSUM bank
            hx_ps = hx_pool.tile([D, H], f32, name=f"hx_ps{h}", tag=f"hx{h}")
            for p in range(h * 4, (h + 1) * 4):
                ps = slice(p * 2 * D, p * 2 * D + D)
                ms = slice(p * 2 * D + D, (p + 1) * 2 * D)
                nc.tensor.matmul(
                    hx_ps[:, (p % 4) * 2 * N : (p % 4 + 1) * 2 * N],
                    in1_sb[:, ps],
                    in1_sb[:, ms],
                    start=True,
                    stop=True,
                )
            s = slice(h * H, (h + 1) * H)
            nc.scalar.copy(hx_sb[:, s], hx_ps[:])  # rounds to dt1

            # stage 2: u-term + h-term + d-term accumulate into one bank
            nc.tensor.matmul(
                u_ps[h][:], wu_view[:, 0:OUT], aft_sb[:, s], start=True, stop=False
            )
            if UPATH == "hilo":
                nc.tensor.matmul(
                    u_ps[h][:],
                    wu_view[:, OUT : 2 * OUT],
                    aft_sb[:, s],
                    start=False,
                    stop=False,
                )
            nc.tensor.matmul(
                u_ps[h][:], wj_view[:], hx_sb[:, s], start=False, stop=False
            )
            if UPATH == "hilo":
                lo_s = slice(BL * N + h * H, BL * N + (h + 1) * H)
                nc.tensor.matmul(
                    u_ps[h][:],
                    wu_view[:, 0:OUT],
                    aft_sb[:, lo_s],
                    start=False,
                    stop=False,
                )
            nc.tensor.matmul(
                u_ps[h][:], wts_view[:], in2_sb[:, s], start=False, stop=True
            )
            for q in range(2):
                qs = slice(h * H + q * H // 2, h * H + (q + 1) * H // 2)
                nc.scalar.activation(
                    outT_sb[:, qs],
                    u_ps[h][:, q * H // 2 : (q + 1) * H // 2],
                    mybir.ActivationFunctionType.Relu,
                    bias=bias_ap,
                    scale=1.0,
                )
                nc.sync.dma_start(outT[:, qs], outT_sb[:, qs])

    nc.compile()
    _cache[key] = nc
    return nc


def _prep_in_maps(diff_vecs, af, A, W, bvec):
    """Host-side shard + relayout. Returns list of per-core input dicts."""
    import ml_dtypes

    bf16 = ml_dtypes.bfloat16
    eye = np.eye(N, dtype=np.float32)
    At = A * (1.0 - eye)[None]  # zero the diagonal: j == i excluded

    wjt = np.ascontiguousarray(W[:, D : 2 * D].T)  # [d, o]
    wts = np.ascontiguousarray(np.tile(W[:, 2 * D : 2 * D + 2].T, (N, 1)))  # wdbig
    wit = np.ascontiguousarray(W[:, 0:D].T)  # [d, o]
    if UPATH == "hilo":
        wu = np.empty((D, 2 * OUT), bf16)
        wu[:, 0:OUT] = wit.astype(bf16)
        wu[:, OUT : 2 * OUT] = (wit - wu[:, 0:OUT].astype(np.float32)).astype(bf16)
        wu_u32 = np.ascontiguousarray(wu).view(np.uint16).view(np.uint32)
    else:
        wu_u32 = wit.view(np.uint32)
    bias63 = ((N - 1.0) * bvec).astype(np.float32)  # [OUT]

    wts_u32 = np.ascontiguousarray(wts.astype(bf16)).view(np.uint16).view(np.uint32)
    if ST1 == "bfloat16":
        wj_u32 = np.ascontiguousarray(wjt.astype(bf16)).view(np.uint16).view(np.uint32)
        wpack = np.empty((D, 129 + OUT // 2 + wj_u32.shape[1]), np.uint32)
        wpack[:, 129 + OUT // 2 :] = wj_u32
        wfr = np.ascontiguousarray(wjt[:, 0:OUT])  # placeholder, unused
    else:
        wpack = np.empty((D, 129 + OUT // 2), np.uint32)
        wfr = np.ascontiguousarray(wjt)
    wpack[:, 0] = bias63.view(np.uint32)
    wpack[:, 1:129] = wu_u32
    wpack[:, 129 : 129 + OUT // 2] = wts_u32

    ndt1 = bf16 if ST1 == "bfloat16" else np.float32
    ndt2 = bf16 if DT2 == "bfloat16" else np.float32

    in_maps = []
    for c in range(NCORES):
        sl = slice(c * BL, (c + 1) * BL)
        af_l = af[sl]  # [BL, N, D]
        At_l = At[sl]  # [BL, i, j]
        diff_l = diff_vecs[sl]  # [BL, i, j, 2]
        At_jbi = At_l.transpose(2, 0, 1)  # [j, b, i]

        af63T = ((N - 1.0) * af_l).transpose(2, 0, 1).reshape(D, BL * N)  # [d,(b,i)]
        if UPATH == "hilo":
            aft_arr = np.empty((D, 2 * BL * N), bf16)
            aft_arr[:, 0 : BL * N] = af63T.astype(bf16)
            aft_arr[:, BL * N :] = (
                af63T - aft_arr[:, 0 : BL * N].astype(np.float32)
            ).astype(bf16)
        else:
            aft_arr = np.ascontiguousarray(af63T)

        # pair layout: [af[2p]; af[2p+1]] stacked on rows, then
        # block-diag [[At^T[2p], 0], [0, At^T[2p+1]]]
        in1 = np.zeros((2 * N, BL // 2, 2 * D), np.float32)
        afT_jbd = af_l.transpose(1, 0, 2)  # [j, b, d]
        in1[0:N, :, 0:D] = afT_jbd[:, 0::2]
        in1[N : 2 * N, :, 0:D] = afT_jbd[:, 1::2]
        in1[0:N, :, D : D + N] = At_jbi[:, 0::2]
        in1[N : 2 * N, :, D + N : 2 * D] = At_jbi[:, 1::2]

        wdiff = At_l[:, :, :, None] * diff_l  # [b, i, j, k]
        in2 = np.ascontiguousarray(
            wdiff.transpose(2, 3, 0, 1).reshape(2 * N, BL * N).astype(ndt2)
        )

        in_maps.append(
            {
                "aft": aft_arr,
                "wpack": wpack,
                "wfr": wfr,
                "in1": np.ascontiguousarray(
                    in1.reshape(2 * N, BL * D).astype(ndt1, copy=False)
                ),
                "in2": in2,
            }
        )
    return in_maps


def _gather(results):
    """[8] x outT[OUT, BL*N] -> out[B, N, OUT]"""
    outT = np.stack([results[c]["outT"] for c in range(NCORES)], axis=0)
    return np.ascontiguousarray(
        outT.reshape(NCORES, OUT, BL, N).transpose(0, 2, 3, 1).reshape(B, N, OUT)
    )


def kernel(**inputs):
    from concourse.bass_utils import run_bass_kernel_spmd

    diff_vecs = np.ascontiguousarray(np.asarray(inputs["diff_vecs"], dtype=np.float32))
    af = np.asarray(inputs["agent_features"], dtype=np.float32)
    A = np.asarray(inputs["A"], dtype=np.float32)
    W = np.asarray(inputs["W"], dtype=np.float32)
    bvec = np.asarray(inputs["b"], dtype=np.float32).reshape(-1)

    nc = _build_nc()
    in_maps = _prep_in_maps(diff_vecs, af, A, W, bvec)
    res = run_bass_kernel_spmd(nc, in_maps, list(range(NCORES))).results
    return diff_vecs, _gather(res)
